# revision 19
# baseline (speedup 1.0000x reference)
"""ChannelMHSA on Trainium2 (Bass/Tile), data-parallel over batch on 8 cores.

Reference computation (per batch b of x [N, C]):
    qkv  = x @ w_qkv                      # [N, 3C], columns ordered (s, h, d)
    q, k, v per head h: [N, D]
    z_h  = k_h^T @ v_h / sqrt(D)          # [D, D]
    A_h  = softmax(z_h, axis=-1)
    out[n, h*D+d] = (A_h @ q_h^T)[d, n]
    y    = out @ w_out                    # [N, C]

b_qkv / b_out are all-zero by construction (see input spec) and are ignored.

Algebra (PE cost = output-free-size rows per 128-deep contraction chunk, so
row count == MACs/128^2 when both tile dims are full):
  Gram trick:   z_h = Wk_h^T (x^T x) Wv_h.  G = x^T x is symmetric (upper
                strips + PE-transpose mirrors).
  Output fuse:  y = x @ M with M = sum_h Wq_h (A_h^T W_out_h).

Perf structure (HW: PE clock ramps to ~2.2GHz only after ~3us of gap-free
execution; every stall resets it, so each batch is built as one long PE
stream and every phase handoff is padded with independent PE work):
  - x^T never touches the PE: a bf16 copy of x (DVE cast) bounces through
    a DRAM scratch and comes back transposed via the DMA xbar
    (dma_start_transpose), on otherwise-idle DMA queues.  y = x @ M is a
    bf16 matmul (M copied out of PSUM as bf16).  Only the softmax-logit
    path (G, GWv, z) stays f32r -- bf16 there costs ~1e-2 rel-err.
  - softmax: one 128-col EXP per head-pair on ACT (the pair's two diag
    blocks are contiguous in its psum slab), row-sums + reciprocals on
    DVE, per-partition scale on ACT.  GWv's psum copies go DVE-only so
    the EXPs start the moment z(0) stops.
  - the softmax wait is covered by real work: next batch's G strip mo=0
    (its x is prefetched ~40us earlier) plus a few zero-matmul warmers;
    batch 0 builds Wq^T there instead.
  - warm-up zero-matmuls run while batch 0's x DMAs land, so the clock
    ramp starts before the first real matmul.
"""

import os
import sys
from contextlib import ExitStack

import numpy as np

for _p in ("/opt/trn_rl_repo", "/opt/pypackages"):
    if _p not in sys.path:
        sys.path.append(_p)

import concourse.bacc as bacc
import concourse.mybir as mybir
import concourse.tile as tile
from concourse import bass_utils, masks

B, N, C = 32, 1024, 768
H, D = 12, 64
P = 128
NCORES = 8
BS = B // NCORES          # batches per core
KC = C // P               # 6 chunks over C
NM = N // P               # 8 chunks over N
NPR = H // 2              # 6 head pairs
F32 = mybir.dt.float32
F32R = mybir.dt.float32r
BF16 = mybir.dt.bfloat16

MM_DT_NAME = os.environ.get("BASS_MM_DT", "f32r")

# free-dim segmentation of a 768-wide output: one full psum bank + half
SEGS = ((0, 512), (512, 256))

# G upper-triangle strips: row-chunk i covers cols >= i*128 (rows 4 and 5
# both take cols 512:768 so no strip is narrower than f32r's 256 minimum)
TRI_SEGS = ((0, 512), (512, 256)), ((128, 384), (512, 256)), \
    ((256, 512),), ((384, 384),), ((512, 256),), ((512, 256),)


def _emit(ctx, tc, mm_dt, x_d, wqkv_d, wo_d, y_d):
    nc = tc.nc

    mdt = mm_dt
    def wcast(ap):       # DRAM-side view for f32r DMAs
        return ap.bitcast(mdt) if mdt is not F32 else ap

    const = ctx.enter_context(tc.tile_pool(name="const", bufs=1))
    work = ctx.enter_context(tc.tile_pool(name="work", bufs=6))
    ps = ctx.enter_context(tc.tile_pool(name="ps", bufs=5, space="PSUM"))
    psz = ctx.enter_context(tc.tile_pool(name="psz", bufs=3, space="PSUM"))

    # zeros16 feeds the warm-up / filler matmuls (nothing reads their psum)
    zeros16 = const.tile([P, 256], BF16, tag="zeros16", name="zeros16")
    nc.vector.memset(zeros16[:], 0.0)

    dummy_ix = [0]

    def dummy_fill(n):
        # keep the PE busy (and its clock ramp alive) across a dependency
        # wait: zero matmuls, psum never read, slot freed on next WAW
        for _ in range(n):
            i = dummy_ix[0]
            dummy_ix[0] += 1
            dps = ps.tile([P, 256], F32, tag="ps", padded_shape=[P, 512],
                          name=f"dum{i}", space="PSUM")
            nc.tensor.matmul(dps[:], zeros16[:, 0:P], zeros16[:],
                             start=True, stop=True)

    ident = const.tile([P, P], F32, tag="ident", name="ident")
    masks.make_identity(nc, ident[:])
    identr = const.tile([P, P], mdt, tag="identr", name="identr")
    nc.vector.tensor_copy(identr[:], ident[:])
    ident16 = const.tile([P, P], BF16, tag="ident16", name="ident16")
    nc.vector.tensor_copy(ident16[:], ident[:])

    # block-diag lhsT tiles for P, one per head pair so the softmax writes
    # for pair pr never wait behind this batch's P matmuls (WAR):
    # off-diag zeros persist, diag blocks rewritten every batch.
    zeros = const.tile([P, P], F32, tag="zeros", name="zeros")
    nc.vector.memset(zeros[:], 0.0)
    a2_tiles = []
    for i in range(NPR):
        a2t = const.tile([P, P], mdt, tag=f"a2_{i}", name=f"a2_{i}")
        nc.vector.tensor_copy(a2t[:], zeros[:])
        a2_tiles.append(a2t)

    # ---- weights ----
    # wkv[p]: w_qkv[pP:(p+1)P, C:3C]; cols 0:C = Wk (z lhsT), C:2C = Wv
    # wo[pr]: w_out rows for head pair pr (rhs of P)
    # WqT[pr]: (w_qkv[:, 0:C])^T rows (h,e) in pair pr (lhsT of M),
    #          built by PE transposes from wq chunks staged in the xin ring.
    wkv, wo, wqT = [], [], []
    for p in range(KC):
        t = const.tile([P, 2 * C], mdt, tag=f"wkv{p}", name=f"wkv{p}")
        wkv.append(t)
    for pr in range(NPR):
        t = const.tile([P, C], mdt, tag=f"wo{pr}", name=f"wo{pr}")
        wo.append(t)
    for pr in range(NPR):
        t = const.tile([P, C], mdt, tag=f"wqT{pr}", name=f"wqT{pr}")
        wqT.append(t)

    def load_x(b):
        xin = []
        for m in range(NM):
            t = work.tile([P, C], mdt, tag="xin", bufs=9, name=f"xin{b}_{m}")
            nc.sync.dma_start(t[:], wcast(x_d[b, m * P:(m + 1) * P, :]))
            xin.append(t)
        return xin

    def cast_x16(b, xin):
        # bf16 copy of x: source for the (cheap) bf16 PE transposes
        x16 = []
        for m in range(NM):
            t = work.tile([P, C], BF16, tag="x16", bufs=8, name=f"x16_{b}_{m}")
            nc.vector.tensor_copy(t[:], xin[m][:])
            x16.append(t)
        return x16

    def emit_xT(b, x16):
        # x^T chunks [C-chunk, N] in bf16 via 48 PE transposes, 4 per copy
        xT = [work.tile([P, N], BF16, tag="xT", bufs=6, name=f"xT{b}_{p}")
              for p in range(KC)]
        for p in range(KC):
            for g in range(2):
                gw = 4 * P
                tp = ps.tile([P, gw], BF16, tag="ps", padded_shape=[P, 512],
                             name=f"xtp{b}_{p}_{g}", space="PSUM")
                for q in range(4):
                    nc.tensor.transpose(
                        tp[:, q * P:(q + 1) * P],
                        x16[g * 4 + q][:, p * P:(p + 1) * P], ident16[:])
                copy_out(xT[p][:, g * 512:(g + 1) * 512], tp[:], p * 2 + g)
        return xT

    # batch-0 x first on the sync queue so compute starts immediately
    xin_next = load_x(0)
    for p in range(KC):
        nc.sync.dma_start(wkv[p][:], wcast(wqkv_d[p * P:(p + 1) * P, C:3 * C]))
    # wq chunks ride the xin ring (same shape); slots recycle for x(1)
    wq_raw = []
    for p in range(KC):
        t = work.tile([P, C], mdt, tag="xin", bufs=9, name=f"wq{p}")
        nc.sync.dma_start(t[:], wcast(wqkv_d[p * P:(p + 1) * P, 0:C]))
        wq_raw.append(t)
    for pr in range(NPR):
        nc.sync.dma_start(wo[pr][:],
                          wcast(wo_d[pr * P:(pr + 1) * P, :]))
    x16_first = cast_x16(0, xin_next)

    def copy_out(dst, src, idx, eng=None):
        # psum->sbuf copies: alternate ACT/DVE unless pinned to one engine
        if eng == "v" or (eng is None and idx % 2 == 1):
            nc.vector.tensor_copy(dst, src)
        else:
            nc.scalar.copy(dst, src)

    def transpose_group(dst, srcs, idx):
        # several 128-col transposes into one psum tile, ONE copy out
        gw = P * len(srcs)
        tp = ps.tile([P, gw], mdt, tag="ps", padded_shape=[P, 512],
                     name=f"tg{idx}", space="PSUM")
        for q, src in enumerate(srcs):
            nc.tensor.transpose(tp[:, q * P:(q + 1) * P], src, identr[:])
        copy_out(dst, tp[:, 0:gw], idx)

    def alloc_G(b):
        return [work.tile([P, C], mdt, tag="G", bufs=6, name=f"G{b}_{mo}")
                for mo in range(KC)]

    def emit_G_seg(G, xin, b, mo, si, eng=None):
        s0, sw = TRI_SEGS[mo][si]
        gps = ps.tile([P, sw], F32, tag="ps", padded_shape=[P, 512],
                      name=f"gps{b}_{mo}_{si}", space="PSUM")
        for m in range(NM):
            nc.tensor.matmul(
                gps[:], xin[m][:, mo * P:(mo + 1) * P],
                xin[m][:, s0:s0 + sw],
                start=(m == 0), stop=(m == NM - 1))
        copy_out(G[mo][:, s0:s0 + sw], gps[:], mo * 2 + si, eng)

    # PE warm-up: start the clock ramp while batch 0's x is still loading
    dummy_fill(10)

    G_pre = None          # next batch's G tiles, strip mo=0 computed early
    for b in range(BS):
        xin = xin_next
        x16 = x16_first if b == 0 else None

        # ---- G = x^T x  [C, C] (contraction over N in 8 chunks) ----
        # batch 0's strip matmuls chase the x DMAs chunk by chunk
        G = G_pre if G_pre is not None else alloc_G(b)
        for mo in range(KC):
            if G_pre is not None and mo == 0:
                continue  # strip 0 ran in the previous batch's softmax slot
            for si in range(len(TRI_SEGS[mo])):
                emit_G_seg(G, xin, b, mo, si, eng="a")
        # mirrors grouped by destination row: one copy per row j
        for j in range(1, KC):
            ii = [i for i in range(j) if (i, j) != (4, 5)]
            if not ii:
                continue
            transpose_group(G[j][:, ii[0] * P:(ii[-1] + 1) * P],
                            [G[i][:, j * P:(j + 1) * P] for i in ii], j)

        # cast after G so the 8 DVE copies trail G's ACT-pinned drains;
        # then prefetch next batch's x (slots free once G + casts consume)
        if x16 is None:
            x16 = cast_x16(b, xin)
        if b + 1 < BS:
            xin_next = load_x(b + 1)

        # ---- GWv = G @ Wv  [C, C] ----
        # descending r gives the mirror copies time to land; copies pinned
        # to DVE so ACT is empty when the softmax EXPs arrive
        GWv = [work.tile([P, C], mdt, tag="GWv", bufs=6, name=f"GWv{b}_{r}")
               for r in range(KC)]
        for r in reversed(range(KC)):
            for si, (s0, sw) in enumerate(SEGS):
                gwps = ps.tile([P, sw], F32, tag="ps", padded_shape=[P, 512],
                               name=f"gwps{b}_{r}_{si}", space="PSUM")
                for k in range(KC):
                    nc.tensor.matmul(
                        gwps[:], G[k][:, r * P:(r + 1) * P],
                        wkv[k][:, C + s0:C + s0 + sw],
                        start=(k == 0), stop=(k == KC - 1))
                copy_out(GWv[r][:, s0:s0 + sw], gwps[:], r * 2 + si, eng="v")

        # ---- z per head pair: z_pair = Wk_pair^T @ GWv 4-head slab ----
        # psum rows 0:64 = head 2pr, 64:128 = head 2pr+1; cols = 4 heads' e
        zps_pair = {}
        for pr in range(NPR):
            q4 = pr // 2
            zps = psz.tile([P, 256], F32, tag="z", name=f"z{b}_{pr}",
                           space="PSUM")
            zps_pair[pr] = zps
            for k in range(KC):
                nc.tensor.matmul(
                    zps[:], wkv[k][:, pr * P:(pr + 1) * P],
                    GWv[k][:, q4 * 256:(q4 + 1) * 256],
                    start=(k == 0), stop=(k == KC - 1))

        # ---- softmax -> scaled A into block-diag lhsT (ACT + DVE) ----
        for pr in range(NPR):
            zps = zps_pair.pop(pr)
            a2 = a2_tiles[pr]
            c0 = (pr % 2) * P          # col offset of the pair's 128 window
            aex = const.tile([P, P], F32, tag="aex", bufs=6,
                             name=f"aex{b}_{pr}")
            nc.scalar.activation(aex[:, :], zps[:, c0:c0 + P],
                                 mybir.ActivationFunctionType.Exp,
                                 bias=0.0, scale=0.125)
            ssum = const.tile([P, 1], F32, tag="ssum", bufs=6,
                              name=f"ss{b}_{pr}")
            rinv = const.tile([P, 1], F32, tag="rinv", bufs=6,
                              name=f"ri{b}_{pr}")
            for j in range(2):
                rb = j * D
                nc.vector.reduce_sum(ssum[rb:rb + D, :],
                                     aex[rb:rb + D, rb:rb + D],
                                     axis=mybir.AxisListType.X)
                nc.vector.reciprocal(rinv[rb:rb + D, :], ssum[rb:rb + D, :])
                nc.scalar.mul(a2[rb:rb + D, rb:rb + D],
                              aex[rb:rb + D, rb:rb + D], rinv[rb:rb + D, :])

        # ---- PE filler while ACT/DVE run the softmax: one continuous
        # stretch of independent work (batch 0: the one-time WqT build;
        # all but the last batch: next batch's G strip mo=0) ----
        xT = emit_xT(b, x16)
        if b == 0:
            # WqT[pr][:, mo] = (Wq[mo-chunk, pr-chunk])^T
            for pr in range(NPR):
                for g, mos in enumerate(((0, 1, 2, 3), (4, 5))):
                    transpose_group(
                        wqT[pr][:, mos[0] * P:(mos[-1] + 1) * P],
                        [wq_raw[mo][:, pr * P:(pr + 1) * P] for mo in mos],
                        pr * 2 + g)
        if b + 1 < BS:
            G_pre = alloc_G(b + 1)
            for si in range(len(TRI_SEGS[0])):
                emit_G_seg(G_pre, xin_next, b + 1, 0, si, eng="v")
        else:
            G_pre = None
        dummy_fill(6)

        # ---- P_pair = (scaled A)^T @ w_out rows of the pair ----
        Pt = [work.tile([P, C], mdt, tag="P", bufs=6, name=f"P{b}_{pr}")
              for pr in range(NPR)]
        for pr in range(NPR):
            for si, (s0, sw) in enumerate(SEGS):
                pps = ps.tile([P, sw], F32, tag="ps", padded_shape=[P, 512],
                              name=f"pps{b}_{pr}_{si}", space="PSUM")
                nc.tensor.matmul(pps[:], a2_tiles[pr][:],
                                 wo[pr][:, s0:s0 + sw],
                                 start=True, stop=True)
                copy_out(Pt[pr][:, s0:s0 + sw], pps[:], pr * 2 + si)

        # ---- M = Wq^T @ P  [C, C], copied out as bf16 (rhs of y) ----
        # accumulation order rotated per mo so each psum's early matmuls
        # use Pt tiles whose copies have already landed
        Mt = [work.tile([P, C], BF16, tag="M", bufs=6, name=f"M{b}_{mo}")
              for mo in range(KC)]
        for mo in range(KC):
            for si, (s0, sw) in enumerate(SEGS):
                mps = ps.tile([P, sw], F32, tag="ps", padded_shape=[P, 512],
                              name=f"mps{b}_{mo}_{si}", space="PSUM")
                for i in range(NPR):
                    pr = (mo + i) % NPR
                    nc.tensor.matmul(
                        mps[:], wqT[pr][:, mo * P:(mo + 1) * P],
                        Pt[pr][:, s0:s0 + sw],
                        start=(i == 0), stop=(i == NPR - 1))
                copy_out(Mt[mo][:, s0:s0 + sw], mps[:], mo * 2 + si)

        # ---- y = x @ M  (bf16 lhsT and rhs, fp32 psum) ----
        for m in range(NM):
            yt = work.tile([P, C], F32, tag="y", bufs=2, name=f"y{b}_{m}")
            for si, (s0, sw) in enumerate(SEGS):
                yps = ps.tile([P, sw], F32, tag="ps", padded_shape=[P, 512],
                              name=f"yps{b}_{m}_{si}", space="PSUM")
                for k in range(KC):
                    nc.tensor.matmul(
                        yps[:], xT[k][:, m * P:(m + 1) * P],
                        Mt[k][:, s0:s0 + sw],
                        start=(k == 0), stop=(k == KC - 1))
                copy_out(yt[:, s0:s0 + sw], yps[:], m * 2 + si)
                # per-segment DMA out: shorter tail after the last copy
                nc.sync.dma_start(y_d[b, m * P:(m + 1) * P, s0:s0 + sw],
                                  yt[:, s0:s0 + sw])


_BUILD_CACHE = {}


def build_program(mm_dt_name=MM_DT_NAME):
    if mm_dt_name in _BUILD_CACHE:
        return _BUILD_CACHE[mm_dt_name]
    mm_dt = F32R if mm_dt_name == "f32r" else F32
    nc = bacc.Bacc("TRN2", target_bir_lowering=False, debug=False,
                   num_devices=NCORES)
    x_d = nc.dram_tensor("x", [BS, N, C], F32, kind="ExternalInput").ap()
    wqkv_d = nc.dram_tensor("w_qkv", [C, 3 * C], F32, kind="ExternalInput").ap()
    wo_d = nc.dram_tensor("w_out", [C, C], F32, kind="ExternalInput").ap()
    y_d = nc.dram_tensor("y", [BS, N, C], F32, kind="ExternalOutput").ap()
    with tile.TileContext(nc) as tc:
        with ExitStack() as ctx:
            _emit(ctx, tc, mm_dt, x_d, wqkv_d, wo_d, y_d)
    nc.compile()
    _BUILD_CACHE[mm_dt_name] = nc
    return nc


def make_in_maps(x, w_qkv, w_out):
    x = np.ascontiguousarray(np.asarray(x, dtype=np.float32))
    w_qkv = np.ascontiguousarray(np.asarray(w_qkv, dtype=np.float32))
    w_out = np.ascontiguousarray(np.asarray(w_out, dtype=np.float32))
    return [
        {"x": x[i * BS:(i + 1) * BS], "w_qkv": w_qkv, "w_out": w_out}
        for i in range(NCORES)
    ]


def kernel(x, w_qkv, b_qkv=None, w_out=None, b_out=None, **_unused):
    nc = build_program()
    in_maps = make_in_maps(x, w_qkv, w_out)
    res = bass_utils.run_bass_kernel_spmd(nc, in_maps,
                                          core_ids=list(range(NCORES)))
    y = np.concatenate([res.results[i]["y"] for i in range(NCORES)], axis=0)
    return np.asarray(y, dtype=np.float32)


# revision 21
# speedup vs baseline: 1.1983x; 1.1983x over previous
"""ChannelMHSA on Trainium2 (Bass/Tile), data-parallel over batch on 8 cores.

Reference computation (per batch b of x [N, C]):
    qkv  = x @ w_qkv                      # [N, 3C], columns ordered (s, h, d)
    q, k, v per head h: [N, D]
    z_h  = k_h^T @ v_h / sqrt(D)          # [D, D]
    A_h  = softmax(z_h, axis=-1)
    out[n, h*D+d] = (A_h @ q_h^T)[d, n]
    y    = out @ w_out                    # [N, C]

b_qkv / b_out are all-zero by construction (see input spec) and are ignored.

Algebra (PE cost = output-free-size rows per 128-deep contraction chunk, so
row count == MACs/128^2 when both tile dims are full):
  Gram trick:   z_h = Wk_h^T (x^T x) Wv_h.  G = x^T x is symmetric (upper
                strips + PE-transpose mirrors).
  Output fuse:  y = x @ M with M = sum_h Wq_h (A_h^T W_out_h).

Perf structure (HW: PE clock ramps to ~2.2GHz only after ~3us of gap-free
execution; every stall resets it, so each batch is built as one long PE
stream and every phase handoff is padded with independent PE work):
  - x^T never touches the PE: a bf16 copy of x (DVE cast) bounces through
    a DRAM scratch and comes back transposed via the DMA xbar
    (dma_start_transpose), on otherwise-idle DMA queues.  y = x @ M is a
    bf16 matmul (M copied out of PSUM as bf16).  Only the softmax-logit
    path (G, GWv, z) stays f32r -- bf16 there costs ~1e-2 rel-err.
  - softmax: one 128-col EXP per head-pair on ACT (the pair's two diag
    blocks are contiguous in its psum slab), row-sums + reciprocals on
    DVE, per-partition scale on ACT.  GWv's psum copies go DVE-only so
    the EXPs start the moment z(0) stops.
  - the softmax wait is covered by real work: next batch's G strip mo=0
    (its x is prefetched ~40us earlier) plus a few zero-matmul warmers;
    batch 0 builds Wq^T there instead.
  - warm-up zero-matmuls run while batch 0's x DMAs land, so the clock
    ramp starts before the first real matmul.
"""

import os
import sys
from contextlib import ExitStack

import numpy as np

for _p in ("/opt/trn_rl_repo", "/opt/pypackages"):
    if _p not in sys.path:
        sys.path.append(_p)

import concourse.bacc as bacc
import concourse.mybir as mybir
import concourse.tile as tile
from concourse import bass_utils, masks

B, N, C = 32, 1024, 768
H, D = 12, 64
P = 128
NCORES = 8
BS = B // NCORES          # batches per core
KC = C // P               # 6 chunks over C
NM = N // P               # 8 chunks over N
NPR = H // 2              # 6 head pairs
F32 = mybir.dt.float32
F32R = mybir.dt.float32r
BF16 = mybir.dt.bfloat16

MM_DT_NAME = os.environ.get("BASS_MM_DT", "f32r")

# free-dim segmentation of a 768-wide output: one full psum bank + half
SEGS = ((0, 512), (512, 256))

# G upper-triangle strips: row-chunk i covers cols >= i*128 (rows 4 and 5
# both take cols 512:768 so no strip is narrower than f32r's 256 minimum)
TRI_SEGS = ((0, 512), (512, 256)), ((128, 384), (512, 256)), \
    ((256, 512),), ((384, 384),), ((512, 256),), ((512, 256),)


def _emit(ctx, tc, mm_dt, x_d, wqkv_d, wo_d, y_d):
    nc = tc.nc

    mdt = mm_dt
    def wcast(ap):       # DRAM-side view for f32r DMAs
        return ap.bitcast(mdt) if mdt is not F32 else ap

    const = ctx.enter_context(tc.tile_pool(name="const", bufs=1))
    work = ctx.enter_context(tc.tile_pool(name="work", bufs=6))
    ps = ctx.enter_context(tc.tile_pool(name="ps", bufs=5, space="PSUM"))
    psz = ctx.enter_context(tc.tile_pool(name="psz", bufs=3, space="PSUM"))

    # zeros16 feeds the warm-up / filler matmuls (nothing reads their psum)
    zeros16 = const.tile([P, 256], BF16, tag="zeros16", name="zeros16")
    nc.vector.memset(zeros16[:], 0.0)

    dummy_ix = [0]

    def dummy_fill(n):
        # keep the PE busy (and its clock ramp alive) across a dependency
        # wait: zero matmuls, psum never read, slot freed on next WAW
        for _ in range(n):
            i = dummy_ix[0]
            dummy_ix[0] += 1
            dps = ps.tile([P, 256], F32, tag="ps", padded_shape=[P, 512],
                          name=f"dum{i}", space="PSUM")
            nc.tensor.matmul(dps[:], zeros16[:, 0:P], zeros16[:],
                             start=True, stop=True)

    ident = const.tile([P, P], F32, tag="ident", name="ident")
    masks.make_identity(nc, ident[:])
    identr = const.tile([P, P], mdt, tag="identr", name="identr")
    nc.vector.tensor_copy(identr[:], ident[:])
    ident16 = const.tile([P, P], BF16, tag="ident16", name="ident16")
    nc.vector.tensor_copy(ident16[:], ident[:])

    # block-diag lhsT tiles for P, one per head pair so the softmax writes
    # for pair pr never wait behind this batch's P matmuls (WAR):
    # off-diag zeros persist, diag blocks rewritten every batch.
    zeros = const.tile([P, P], F32, tag="zeros", name="zeros")
    nc.vector.memset(zeros[:], 0.0)
    a2_tiles = []
    for i in range(NPR):
        a2t = const.tile([P, P], mdt, tag=f"a2_{i}", name=f"a2_{i}")
        nc.vector.tensor_copy(a2t[:], zeros[:])
        a2_tiles.append(a2t)

    # ---- weights ----
    # wkv[p]: w_qkv[pP:(p+1)P, C:3C]; cols 0:C = Wk (z lhsT), C:2C = Wv
    # wo[pr]: w_out rows for head pair pr (rhs of P)
    # WqT[pr]: (w_qkv[:, 0:C])^T rows (h,e) in pair pr (lhsT of M),
    #          built by PE transposes from wq chunks staged in the xin ring.
    wkv, wo, wqT = [], [], []
    for p in range(KC):
        t = const.tile([P, 2 * C], mdt, tag=f"wkv{p}", name=f"wkv{p}")
        wkv.append(t)
    for pr in range(NPR):
        t = const.tile([P, C], mdt, tag=f"wo{pr}", name=f"wo{pr}")
        wo.append(t)
    for pr in range(NPR):
        t = const.tile([P, C], BF16, tag=f"wqT{pr}", name=f"wqT{pr}")
        wqT.append(t)

    def load_x(b):
        xin = []
        for m in range(NM):
            t = work.tile([P, C], mdt, tag="xin", bufs=9, name=f"xin{b}_{m}")
            nc.sync.dma_start(t[:], wcast(x_d[b, m * P:(m + 1) * P, :]))
            xin.append(t)
        return xin

    def alloc_x16(b):
        return [work.tile([P, C], BF16, tag="x16", bufs=8,
                          name=f"x16_{b}_{m}") for m in range(NM)]

    def cast_x16_chunk(x16, xin, m):
        # bf16 copy of one x chunk, engines alternated so neither ACT nor
        # DVE sees a long cast backlog ahead of its psum-copy duties
        if m % 2 == 0:
            nc.vector.tensor_copy(x16[m][:], xin[m][:])
        else:
            nc.scalar.copy(x16[m][:], xin[m][:])

    def cast_x16(b, xin):
        x16 = alloc_x16(b)
        for m in range(NM):
            cast_x16_chunk(x16, xin, m)
        return x16

    def emit_xT(b, x16):
        # x^T chunks [C-chunk, N] in bf16 via 48 PE transposes, 4 per copy
        xT = [work.tile([P, N], BF16, tag="xT", bufs=6, name=f"xT{b}_{p}")
              for p in range(KC)]
        for p in range(KC):
            for g in range(2):
                gw = 4 * P
                tp = ps.tile([P, gw], BF16, tag="ps", padded_shape=[P, 512],
                             name=f"xtp{b}_{p}_{g}", space="PSUM")
                for q in range(4):
                    nc.tensor.transpose(
                        tp[:, q * P:(q + 1) * P],
                        x16[g * 4 + q][:, p * P:(p + 1) * P], ident16[:])
                copy_out(xT[p][:, g * 512:(g + 1) * 512], tp[:], p * 2 + g)
        return xT

    # batch-0 x first on the sync queue so compute starts immediately
    xin_next = load_x(0)
    for p in range(KC):
        nc.sync.dma_start(wkv[p][:], wcast(wqkv_d[p * P:(p + 1) * P, C:3 * C]))
    # wq chunks ride the xin ring (same shape); slots recycle for x(1)
    wq_raw = []
    for p in range(KC):
        t = work.tile([P, C], mdt, tag="xin", bufs=9, name=f"wq{p}")
        nc.sync.dma_start(t[:], wcast(wqkv_d[p * P:(p + 1) * P, 0:C]))
        wq_raw.append(t)
    for pr in range(NPR):
        nc.sync.dma_start(wo[pr][:],
                          wcast(wo_d[pr * P:(pr + 1) * P, :]))
    x16_first = cast_x16(0, xin_next)

    def copy_out(dst, src, idx, eng=None):
        # psum->sbuf copies: alternate ACT/DVE unless pinned to one engine
        if eng == "v" or (eng is None and idx % 2 == 1):
            nc.vector.tensor_copy(dst, src)
        else:
            nc.scalar.copy(dst, src)

    def transpose_group(dst, srcs, idx):
        # several 128-col transposes into one psum tile, ONE copy out
        gw = P * len(srcs)
        tp = ps.tile([P, gw], mdt, tag="ps", padded_shape=[P, 512],
                     name=f"tg{idx}", space="PSUM")
        for q, src in enumerate(srcs):
            nc.tensor.transpose(tp[:, q * P:(q + 1) * P], src, identr[:])
        copy_out(dst, tp[:, 0:gw], idx)

    def alloc_G(b):
        return [work.tile([P, C], mdt, tag="G", bufs=6, name=f"G{b}_{mo}")
                for mo in range(KC)]

    def emit_G_seg(G, xin, b, mo, si, eng=None):
        s0, sw = TRI_SEGS[mo][si]
        gps = ps.tile([P, sw], F32, tag="ps", padded_shape=[P, 512],
                      name=f"gps{b}_{mo}_{si}", space="PSUM")
        for m in range(NM):
            nc.tensor.matmul(
                gps[:], xin[m][:, mo * P:(mo + 1) * P],
                xin[m][:, s0:s0 + sw],
                start=(m == 0), stop=(m == NM - 1))
        copy_out(G[mo][:, s0:s0 + sw], gps[:], mo * 2 + si, eng)

    # PE warm-up: start the clock ramp while batch 0's x is still loading
    dummy_fill(10)

    G_pre = None          # next batch's G tiles, strip mo=0 computed early
    for b in range(BS):
        xin = xin_next
        if b == 0:
            x16 = x16_first
        else:
            x16 = alloc_x16(b)
            cast_x16_chunk(x16, xin, 0)
            cast_x16_chunk(x16, xin, 1)

        # ---- G = x^T x  [C, C] (contraction over N in 8 chunks) ----
        # batch 0's strip matmuls chase the x DMAs chunk by chunk
        G = G_pre if G_pre is not None else alloc_G(b)
        cast_m = 2
        for mo in range(KC):
            if G_pre is not None and mo == 0:
                continue  # strip 0 ran in the previous batch's softmax slot
            for si in range(len(TRI_SEGS[mo])):
                emit_G_seg(G, xin, b, mo, si)
            if b > 0:
                for _ in range(2):
                    if cast_m < NM:
                        cast_x16_chunk(x16, xin, cast_m)
                        cast_m += 1
        # mirrors grouped by destination row: one copy per row j
        for j in range(1, KC):
            ii = [i for i in range(j) if (i, j) != (4, 5)]
            if not ii:
                continue
            transpose_group(G[j][:, ii[0] * P:(ii[-1] + 1) * P],
                            [G[i][:, j * P:(j + 1) * P] for i in ii], j)

        # prefetch next batch's x: slots free once G + casts consume
        if b + 1 < BS:
            xin_next = load_x(b + 1)

        # ---- GWv = G @ Wv  [C, C] ----
        # descending r gives the mirror copies time to land; copies pinned
        # to DVE so ACT is empty when the softmax EXPs arrive
        GWv = [work.tile([P, C], mdt, tag="GWv", bufs=6, name=f"GWv{b}_{r}")
               for r in range(KC)]
        for r in reversed(range(KC)):
            for si, (s0, sw) in enumerate(SEGS):
                gwps = ps.tile([P, sw], F32, tag="ps", padded_shape=[P, 512],
                               name=f"gwps{b}_{r}_{si}", space="PSUM")
                for k in range(KC):
                    nc.tensor.matmul(
                        gwps[:], G[k][:, r * P:(r + 1) * P],
                        wkv[k][:, C + s0:C + s0 + sw],
                        start=(k == 0), stop=(k == KC - 1))
                copy_out(GWv[r][:, s0:s0 + sw], gwps[:], r * 2 + si, eng="v")

        # ---- z per head pair: z_pair = Wk_pair^T @ GWv 4-head slab ----
        # psum rows 0:64 = head 2pr, 64:128 = head 2pr+1; cols = 4 heads' e
        zps_pair = {}
        for pr in range(NPR):
            q4 = pr // 2
            zps = psz.tile([P, 256], F32, tag="z", name=f"z{b}_{pr}",
                           space="PSUM")
            zps_pair[pr] = zps
            for k in range(KC):
                nc.tensor.matmul(
                    zps[:], wkv[k][:, pr * P:(pr + 1) * P],
                    GWv[k][:, q4 * 256:(q4 + 1) * 256],
                    start=(k == 0), stop=(k == KC - 1))

        # ---- softmax -> scaled A into block-diag lhsT (ACT + DVE) ----
        for pr in range(NPR):
            zps = zps_pair.pop(pr)
            a2 = a2_tiles[pr]
            c0 = (pr % 2) * P          # col offset of the pair's 128 window
            aex = const.tile([P, P], F32, tag="aex", bufs=6,
                             name=f"aex{b}_{pr}")
            nc.scalar.activation(aex[:, :], zps[:, c0:c0 + P],
                                 mybir.ActivationFunctionType.Exp,
                                 bias=0.0, scale=0.125)
            ssum = const.tile([P, 1], F32, tag="ssum", bufs=6,
                              name=f"ss{b}_{pr}")
            rinv = const.tile([P, 1], F32, tag="rinv", bufs=6,
                              name=f"ri{b}_{pr}")
            for j in range(2):
                rb = j * D
                nc.vector.reduce_sum(ssum[rb:rb + D, :],
                                     aex[rb:rb + D, rb:rb + D],
                                     axis=mybir.AxisListType.X)
                nc.vector.reciprocal(rinv[rb:rb + D, :], ssum[rb:rb + D, :])
                nc.scalar.mul(a2[rb:rb + D, rb:rb + D],
                              aex[rb:rb + D, rb:rb + D], rinv[rb:rb + D, :])

        # ---- PE filler while ACT/DVE run the softmax: one continuous
        # stretch of independent work (batch 0: the one-time WqT build;
        # all but the last batch: next batch's G strip mo=0) ----
        xT = emit_xT(b, x16)
        if b == 0:
            # WqT[pr][:, mo] = (Wq[mo-chunk, pr-chunk])^T, in bf16 (M's
            # lhsT): cast wq once on gpsimd, transpose at the bf16 rate,
            # copies pinned DVE (ACT is mid-softmax here)
            wq16 = []
            for mo in range(KC):
                t = work.tile([P, C], BF16, tag="x16", bufs=8,
                              name=f"wq16_{mo}")
                nc.vector.tensor_copy(t[:], wq_raw[mo][:])
                wq16.append(t)
            for pr in range(NPR):
                for g, mos in enumerate(((0, 1, 2, 3), (4, 5))):
                    gw = P * len(mos)
                    tp = ps.tile([P, gw], BF16, tag="ps",
                                 padded_shape=[P, 512],
                                 name=f"wqtp{pr}_{g}", space="PSUM")
                    for q, mo in enumerate(mos):
                        nc.tensor.transpose(
                            tp[:, q * P:(q + 1) * P],
                            wq16[mo][:, pr * P:(pr + 1) * P], ident16[:])
                    copy_out(wqT[pr][:, mos[0] * P:(mos[-1] + 1) * P],
                             tp[:, 0:gw], 0, eng="v")
        if b + 1 < BS:
            G_pre = alloc_G(b + 1)
            for si in range(len(TRI_SEGS[0])):
                emit_G_seg(G_pre, xin_next, b + 1, 0, si, eng="v")
        else:
            G_pre = None
        dummy_fill(6)

        # ---- P_pair = (scaled A)^T @ w_out rows of the pair ----
        Pt = [work.tile([P, C], BF16, tag="P", bufs=6, name=f"P{b}_{pr}")
              for pr in range(NPR)]
        for pr in range(NPR):
            for si, (s0, sw) in enumerate(SEGS):
                pps = ps.tile([P, sw], F32, tag="ps", padded_shape=[P, 512],
                              name=f"pps{b}_{pr}_{si}", space="PSUM")
                nc.tensor.matmul(pps[:], a2_tiles[pr][:],
                                 wo[pr][:, s0:s0 + sw],
                                 start=True, stop=True)
                copy_out(Pt[pr][:, s0:s0 + sw], pps[:], pr * 2 + si)

        # ---- M = Wq^T @ P  [C, C], copied out as bf16 (rhs of y) ----
        # accumulation order rotated per mo so each psum's early matmuls
        # use Pt tiles whose copies have already landed
        Mt = [work.tile([P, C], BF16, tag="M", bufs=6, name=f"M{b}_{mo}")
              for mo in range(KC)]
        for mo in range(KC):
            for si, (s0, sw) in enumerate(SEGS):
                mps = ps.tile([P, sw], F32, tag="ps", padded_shape=[P, 512],
                              name=f"mps{b}_{mo}_{si}", space="PSUM")
                for i in range(NPR):
                    pr = (mo + i) % NPR
                    nc.tensor.matmul(
                        mps[:], wqT[pr][:, mo * P:(mo + 1) * P],
                        Pt[pr][:, s0:s0 + sw],
                        start=(i == 0), stop=(i == NPR - 1))
                copy_out(Mt[mo][:, s0:s0 + sw], mps[:], mo * 2 + si)

        # ---- y = x @ M  (bf16 lhsT and rhs, fp32 psum) ----
        for m in range(NM):
            yt = work.tile([P, C], F32, tag="y", bufs=2, name=f"y{b}_{m}")
            for si, (s0, sw) in enumerate(SEGS):
                yps = ps.tile([P, sw], F32, tag="ps", padded_shape=[P, 512],
                              name=f"yps{b}_{m}_{si}", space="PSUM")
                for k in range(KC):
                    nc.tensor.matmul(
                        yps[:], xT[k][:, m * P:(m + 1) * P],
                        Mt[k][:, s0:s0 + sw],
                        start=(k == 0), stop=(k == KC - 1))
                copy_out(yt[:, s0:s0 + sw], yps[:], m * 2 + si)
                # per-segment DMA out: shorter tail after the last copy
                nc.sync.dma_start(y_d[b, m * P:(m + 1) * P, s0:s0 + sw],
                                  yt[:, s0:s0 + sw])


_BUILD_CACHE = {}


def build_program(mm_dt_name=MM_DT_NAME):
    if mm_dt_name in _BUILD_CACHE:
        return _BUILD_CACHE[mm_dt_name]
    mm_dt = F32R if mm_dt_name == "f32r" else F32
    nc = bacc.Bacc("TRN2", target_bir_lowering=False, debug=False,
                   num_devices=NCORES)
    x_d = nc.dram_tensor("x", [BS, N, C], F32, kind="ExternalInput").ap()
    wqkv_d = nc.dram_tensor("w_qkv", [C, 3 * C], F32, kind="ExternalInput").ap()
    wo_d = nc.dram_tensor("w_out", [C, C], F32, kind="ExternalInput").ap()
    y_d = nc.dram_tensor("y", [BS, N, C], F32, kind="ExternalOutput").ap()
    with tile.TileContext(nc) as tc:
        with ExitStack() as ctx:
            _emit(ctx, tc, mm_dt, x_d, wqkv_d, wo_d, y_d)
    nc.compile()
    _BUILD_CACHE[mm_dt_name] = nc
    return nc


def make_in_maps(x, w_qkv, w_out):
    x = np.ascontiguousarray(np.asarray(x, dtype=np.float32))
    w_qkv = np.ascontiguousarray(np.asarray(w_qkv, dtype=np.float32))
    w_out = np.ascontiguousarray(np.asarray(w_out, dtype=np.float32))
    return [
        {"x": x[i * BS:(i + 1) * BS], "w_qkv": w_qkv, "w_out": w_out}
        for i in range(NCORES)
    ]


def kernel(x, w_qkv, b_qkv=None, w_out=None, b_out=None, **_unused):
    nc = build_program()
    in_maps = make_in_maps(x, w_qkv, w_out)
    res = bass_utils.run_bass_kernel_spmd(nc, in_maps,
                                          core_ids=list(range(NCORES)))
    y = np.concatenate([res.results[i]["y"] for i in range(NCORES)], axis=0)
    return np.asarray(y, dtype=np.float32)


# revision 24
# speedup vs baseline: 1.2089x; 1.0088x over previous
"""ChannelMHSA on Trainium2 (Bass/Tile), data-parallel over batch on 8 cores.

Reference computation (per batch b of x [N, C]):
    qkv  = x @ w_qkv                      # [N, 3C], columns ordered (s, h, d)
    q, k, v per head h: [N, D]
    z_h  = k_h^T @ v_h / sqrt(D)          # [D, D]
    A_h  = softmax(z_h, axis=-1)
    out[n, h*D+d] = (A_h @ q_h^T)[d, n]
    y    = out @ w_out                    # [N, C]

b_qkv / b_out are all-zero by construction (see input spec) and are ignored.

Algebra (PE cost = output-free-size rows per 128-deep contraction chunk, so
row count == MACs/128^2 when both tile dims are full):
  Gram trick:   z_h = Wk_h^T (x^T x) Wv_h.  G = x^T x is symmetric (upper
                strips + PE-transpose mirrors).
  Output fuse:  y = x @ M with M = sum_h Wq_h (A_h^T W_out_h).

Perf structure (HW: PE clock ramps to ~2.2GHz only after ~3us of gap-free
execution; every stall resets it, so each batch is built as one long PE
stream and every phase handoff is padded with independent PE work):
  - x^T never touches the PE: a bf16 copy of x (DVE cast) bounces through
    a DRAM scratch and comes back transposed via the DMA xbar
    (dma_start_transpose), on otherwise-idle DMA queues.  y = x @ M is a
    bf16 matmul (M copied out of PSUM as bf16).  Only the softmax-logit
    path (G, GWv, z) stays f32r -- bf16 there costs ~1e-2 rel-err.
  - softmax: one 128-col EXP per head-pair on ACT (the pair's two diag
    blocks are contiguous in its psum slab), row-sums + reciprocals on
    DVE, per-partition scale on ACT.  GWv's psum copies go DVE-only so
    the EXPs start the moment z(0) stops.
  - the softmax wait is covered by real work: next batch's G strip mo=0
    (its x is prefetched ~40us earlier) plus a few zero-matmul warmers;
    batch 0 builds Wq^T there instead.
  - warm-up zero-matmuls run while batch 0's x DMAs land, so the clock
    ramp starts before the first real matmul.
"""

import os
import sys
from contextlib import ExitStack

import numpy as np

for _p in ("/opt/trn_rl_repo", "/opt/pypackages"):
    if _p not in sys.path:
        sys.path.append(_p)

import concourse.bacc as bacc
import concourse.mybir as mybir
import concourse.tile as tile
from concourse import bass_utils, masks

B, N, C = 32, 1024, 768
H, D = 12, 64
P = 128
NCORES = 8
BS = B // NCORES          # batches per core
KC = C // P               # 6 chunks over C
NM = N // P               # 8 chunks over N
NPR = H // 2              # 6 head pairs
F32 = mybir.dt.float32
F32R = mybir.dt.float32r
BF16 = mybir.dt.bfloat16

MM_DT_NAME = os.environ.get("BASS_MM_DT", "f32r")

# free-dim segmentation of a 768-wide output: one full psum bank + half
SEGS = ((0, 512), (512, 256))

# G upper-triangle strips: row-chunk i covers cols >= i*128 (rows 4 and 5
# both take cols 512:768 so no strip is narrower than f32r's 256 minimum)
TRI_SEGS = ((0, 512), (512, 256)), ((128, 384), (512, 256)), \
    ((256, 512),), ((384, 384),), ((512, 256),), ((512, 256),)


def _emit(ctx, tc, mm_dt, x_d, wqkv_d, wo_d, y_d):
    nc = tc.nc

    mdt = mm_dt
    def wcast(ap):       # DRAM-side view for f32r DMAs
        return ap.bitcast(mdt) if mdt is not F32 else ap

    const = ctx.enter_context(tc.tile_pool(name="const", bufs=1))
    work = ctx.enter_context(tc.tile_pool(name="work", bufs=6))
    ps = ctx.enter_context(tc.tile_pool(name="ps", bufs=5, space="PSUM"))
    psz = ctx.enter_context(tc.tile_pool(name="psz", bufs=3, space="PSUM"))

    # zeros16 feeds the warm-up / filler matmuls (nothing reads their psum)
    zeros16 = const.tile([P, 256], BF16, tag="zeros16", name="zeros16")
    nc.vector.memset(zeros16[:], 0.0)

    dummy_ix = [0]

    def dummy_fill(n):
        # keep the PE busy (and its clock ramp alive) across a dependency
        # wait: zero matmuls, psum never read, slot freed on next WAW
        for _ in range(n):
            i = dummy_ix[0]
            dummy_ix[0] += 1
            dps = ps.tile([P, 256], F32, tag="ps", padded_shape=[P, 512],
                          name=f"dum{i}", space="PSUM")
            nc.tensor.matmul(dps[:], zeros16[:, 0:P], zeros16[:],
                             start=True, stop=True)

    ident = const.tile([P, P], F32, tag="ident", name="ident")
    masks.make_identity(nc, ident[:])
    identr = const.tile([P, P], mdt, tag="identr", name="identr")
    nc.vector.tensor_copy(identr[:], ident[:])
    ident16 = const.tile([P, P], BF16, tag="ident16", name="ident16")
    nc.vector.tensor_copy(ident16[:], ident[:])

    # block-diag lhsT tiles for P, one per head pair so the softmax writes
    # for pair pr never wait behind this batch's P matmuls (WAR):
    # off-diag zeros persist, diag blocks rewritten every batch.
    zeros = const.tile([P, P], F32, tag="zeros", name="zeros")
    nc.vector.memset(zeros[:], 0.0)
    a2_tiles = []
    for i in range(NPR):
        a2t = const.tile([P, P], mdt, tag=f"a2_{i}", name=f"a2_{i}")
        nc.vector.tensor_copy(a2t[:], zeros[:])
        a2_tiles.append(a2t)

    # ---- weights ----
    # wkv[p]: w_qkv[pP:(p+1)P, C:3C]; cols 0:C = Wk (z lhsT), C:2C = Wv
    # wo[pr]: w_out rows for head pair pr (rhs of P)
    # WqT[pr]: (w_qkv[:, 0:C])^T rows (h,e) in pair pr (lhsT of M),
    #          built by PE transposes from wq chunks staged in the xin ring.
    wkv, wo, wqT = [], [], []
    for p in range(KC):
        t = const.tile([P, 2 * C], mdt, tag=f"wkv{p}", name=f"wkv{p}")
        wkv.append(t)
    for pr in range(NPR):
        t = const.tile([P, C], mdt, tag=f"wo{pr}", name=f"wo{pr}")
        wo.append(t)
    for pr in range(NPR):
        t = const.tile([P, C], BF16, tag=f"wqT{pr}", name=f"wqT{pr}")
        wqT.append(t)

    def load_x(b):
        xin = []
        for m in range(NM):
            t = work.tile([P, C], mdt, tag="xin", bufs=9, name=f"xin{b}_{m}")
            nc.sync.dma_start(t[:], wcast(x_d[b, m * P:(m + 1) * P, :]))
            xin.append(t)
        return xin

    def alloc_x16(b):
        return [work.tile([P, C], BF16, tag="x16", bufs=8,
                          name=f"x16_{b}_{m}") for m in range(NM)]

    def cast_x16_chunk(x16, xin, m):
        # bf16 copy of one x chunk on ACT: it has no copy duties between
        # the G strips and the softmax EXPs (GWv copies are DVE-pinned)
        nc.scalar.copy(x16[m][:], xin[m][:])

    def cast_x16(b, xin):
        x16 = alloc_x16(b)
        for m in range(NM):
            cast_x16_chunk(x16, xin, m)
        return x16

    def emit_xT(b, x16):
        # x^T chunks [C-chunk, N] in bf16 via 48 PE transposes, 4 per copy
        xT = [work.tile([P, N], BF16, tag="xT", bufs=6, name=f"xT{b}_{p}")
              for p in range(KC)]
        for p in range(KC):
            for g in range(2):
                gw = 4 * P
                tp = ps.tile([P, gw], BF16, tag="ps", padded_shape=[P, 512],
                             name=f"xtp{b}_{p}_{g}", space="PSUM")
                for q in range(4):
                    nc.tensor.transpose(
                        tp[:, q * P:(q + 1) * P],
                        x16[g * 4 + q][:, p * P:(p + 1) * P], ident16[:])
                copy_out(xT[p][:, g * 512:(g + 1) * 512], tp[:], p * 2 + g)
        return xT

    # batch-0 x first on the sync queue so compute starts immediately
    xin_next = load_x(0)
    for p in range(KC):
        nc.sync.dma_start(wkv[p][:], wcast(wqkv_d[p * P:(p + 1) * P, C:3 * C]))
    # wq chunks ride the xin ring (same shape); slots recycle for x(1)
    wq_raw = []
    for p in range(KC):
        t = work.tile([P, C], mdt, tag="xin", bufs=9, name=f"wq{p}")
        nc.sync.dma_start(t[:], wcast(wqkv_d[p * P:(p + 1) * P, 0:C]))
        wq_raw.append(t)
    for pr in range(NPR):
        nc.sync.dma_start(wo[pr][:],
                          wcast(wo_d[pr * P:(pr + 1) * P, :]))
    x16_first = cast_x16(0, xin_next)

    def copy_out(dst, src, idx, eng=None):
        # psum->sbuf copies: alternate ACT/DVE unless pinned to one engine
        if eng == "v" or (eng is None and idx % 2 == 1):
            nc.vector.tensor_copy(dst, src)
        else:
            nc.scalar.copy(dst, src)

    def transpose_group(dst, srcs, idx):
        # several 128-col transposes into one psum tile, ONE copy out
        gw = P * len(srcs)
        tp = ps.tile([P, gw], mdt, tag="ps", padded_shape=[P, 512],
                     name=f"tg{idx}", space="PSUM")
        for q, src in enumerate(srcs):
            nc.tensor.transpose(tp[:, q * P:(q + 1) * P], src, identr[:])
        copy_out(dst, tp[:, 0:gw], idx)

    def alloc_G(b):
        return [work.tile([P, C], mdt, tag="G", bufs=6, name=f"G{b}_{mo}")
                for mo in range(KC)]

    def emit_G_seg(G, xin, b, mo, si, eng=None):
        s0, sw = TRI_SEGS[mo][si]
        gps = ps.tile([P, sw], F32, tag="ps", padded_shape=[P, 512],
                      name=f"gps{b}_{mo}_{si}", space="PSUM")
        for m in range(NM):
            nc.tensor.matmul(
                gps[:], xin[m][:, mo * P:(mo + 1) * P],
                xin[m][:, s0:s0 + sw],
                start=(m == 0), stop=(m == NM - 1))
        copy_out(G[mo][:, s0:s0 + sw], gps[:], mo * 2 + si, eng)

    # PE warm-up: start the clock ramp while batch 0's x is still loading
    dummy_fill(10)

    G_pre = None          # next batch's G tiles, strip mo=0 computed early
    PRE_MO = 2            # strips mo<PRE_MO run in the prior filler
    for b in range(BS):
        xin = xin_next
        x16 = x16_first if b == 0 else alloc_x16(b)

        # ---- G = x^T x  [C, C] (contraction over N in 8 chunks) ----
        # batch 0's strip matmuls chase the x DMAs chunk by chunk
        G = G_pre if G_pre is not None else alloc_G(b)
        for mo in range(KC):
            if G_pre is not None and mo < PRE_MO:
                continue  # ran in the previous batch's softmax slot
            for si in range(len(TRI_SEGS[mo])):
                emit_G_seg(G, xin, b, mo, si)
        if b > 0:
            for m in range(NM):
                cast_x16_chunk(x16, xin, m)
        # mirrors grouped by destination row: one copy per row j
        for j in range(1, KC):
            ii = [i for i in range(j) if (i, j) != (4, 5)]
            if not ii:
                continue
            transpose_group(G[j][:, ii[0] * P:(ii[-1] + 1) * P],
                            [G[i][:, j * P:(j + 1) * P] for i in ii], j)

        # prefetch next batch's x: slots free once G + casts consume
        if b + 1 < BS:
            xin_next = load_x(b + 1)

        # ---- GWv = G @ Wv  [C, C] ----
        # descending r gives the mirror copies time to land; copies pinned
        # to DVE so ACT is empty when the softmax EXPs arrive
        GWv = [work.tile([P, C], mdt, tag="GWv", bufs=6, name=f"GWv{b}_{r}")
               for r in range(KC)]
        for r in reversed(range(KC)):
            for si, (s0, sw) in enumerate(SEGS):
                gwps = ps.tile([P, sw], F32, tag="ps", padded_shape=[P, 512],
                               name=f"gwps{b}_{r}_{si}", space="PSUM")
                for k in range(KC):
                    nc.tensor.matmul(
                        gwps[:], G[k][:, r * P:(r + 1) * P],
                        wkv[k][:, C + s0:C + s0 + sw],
                        start=(k == 0), stop=(k == KC - 1))
                copy_out(GWv[r][:, s0:s0 + sw], gwps[:], r * 2 + si, eng="v")

        # ---- z per head pair: z_pair = Wk_pair^T @ GWv 4-head slab ----
        # psum rows 0:64 = head 2pr, 64:128 = head 2pr+1; cols = 4 heads' e
        zps_pair = {}
        for pr in range(NPR):
            q4 = pr // 2
            zps = psz.tile([P, 256], F32, tag="z", name=f"z{b}_{pr}",
                           space="PSUM")
            zps_pair[pr] = zps
            for k in range(KC):
                nc.tensor.matmul(
                    zps[:], wkv[k][:, pr * P:(pr + 1) * P],
                    GWv[k][:, q4 * 256:(q4 + 1) * 256],
                    start=(k == 0), stop=(k == KC - 1))

        # ---- softmax -> scaled A into block-diag lhsT (ACT + DVE) ----
        for pr in range(NPR):
            zps = zps_pair.pop(pr)
            a2 = a2_tiles[pr]
            c0 = (pr % 2) * P          # col offset of the pair's 128 window
            aex = const.tile([P, P], F32, tag="aex", bufs=6,
                             name=f"aex{b}_{pr}")
            nc.scalar.activation(aex[:, :], zps[:, c0:c0 + P],
                                 mybir.ActivationFunctionType.Exp,
                                 bias=0.0, scale=0.125)
            ssum = const.tile([P, 1], F32, tag="ssum", bufs=6,
                              name=f"ss{b}_{pr}")
            rinv = const.tile([P, 1], F32, tag="rinv", bufs=6,
                              name=f"ri{b}_{pr}")
            for j in range(2):
                rb = j * D
                nc.vector.reduce_sum(ssum[rb:rb + D, :],
                                     aex[rb:rb + D, rb:rb + D],
                                     axis=mybir.AxisListType.X)
                nc.vector.reciprocal(rinv[rb:rb + D, :], ssum[rb:rb + D, :])
                nc.scalar.mul(a2[rb:rb + D, rb:rb + D],
                              aex[rb:rb + D, rb:rb + D], rinv[rb:rb + D, :])

        # ---- PE filler while ACT/DVE run the softmax: one continuous
        # stretch of independent work (batch 0: the one-time WqT build;
        # all but the last batch: next batch's G strip mo=0) ----
        xT = emit_xT(b, x16)
        if b == 0:
            # WqT[pr][:, mo] = (Wq[mo-chunk, pr-chunk])^T, in bf16 (M's
            # lhsT): cast wq once on gpsimd, transpose at the bf16 rate,
            # copies pinned DVE (ACT is mid-softmax here)
            wq16 = []
            for mo in range(KC):
                t = work.tile([P, C], BF16, tag="x16", bufs=8,
                              name=f"wq16_{mo}")
                nc.vector.tensor_copy(t[:], wq_raw[mo][:])
                wq16.append(t)
            for pr in range(NPR):
                for g, mos in enumerate(((0, 1, 2, 3), (4, 5))):
                    gw = P * len(mos)
                    tp = ps.tile([P, gw], BF16, tag="ps",
                                 padded_shape=[P, 512],
                                 name=f"wqtp{pr}_{g}", space="PSUM")
                    for q, mo in enumerate(mos):
                        nc.tensor.transpose(
                            tp[:, q * P:(q + 1) * P],
                            wq16[mo][:, pr * P:(pr + 1) * P], ident16[:])
                    copy_out(wqT[pr][:, mos[0] * P:(mos[-1] + 1) * P],
                             tp[:, 0:gw], 0, eng="v")
        if b + 1 < BS:
            G_pre = alloc_G(b + 1)
            for mo in range(PRE_MO):
                for si in range(len(TRI_SEGS[mo])):
                    emit_G_seg(G_pre, xin_next, b + 1, mo, si, eng="v")
        else:
            G_pre = None
        dummy_fill(3)

        # ---- P_pair = (scaled A)^T @ w_out rows of the pair ----
        Pt = [work.tile([P, C], BF16, tag="P", bufs=6, name=f"P{b}_{pr}")
              for pr in range(NPR)]
        for pr in range(NPR):
            for si, (s0, sw) in enumerate(SEGS):
                pps = ps.tile([P, sw], F32, tag="ps", padded_shape=[P, 512],
                              name=f"pps{b}_{pr}_{si}", space="PSUM")
                nc.tensor.matmul(pps[:], a2_tiles[pr][:],
                                 wo[pr][:, s0:s0 + sw],
                                 start=True, stop=True)
                copy_out(Pt[pr][:, s0:s0 + sw], pps[:], pr * 2 + si)

        # ---- M = Wq^T @ P  [C, C], copied out as bf16 (rhs of y) ----
        # accumulation order rotated per mo so each psum's early matmuls
        # use Pt tiles whose copies have already landed
        Mt = [work.tile([P, C], BF16, tag="M", bufs=6, name=f"M{b}_{mo}")
              for mo in range(KC)]
        for mo in range(KC):
            for si, (s0, sw) in enumerate(SEGS):
                mps = ps.tile([P, sw], F32, tag="ps", padded_shape=[P, 512],
                              name=f"mps{b}_{mo}_{si}", space="PSUM")
                for i in range(NPR):
                    pr = (mo + i) % NPR
                    nc.tensor.matmul(
                        mps[:], wqT[pr][:, mo * P:(mo + 1) * P],
                        Pt[pr][:, s0:s0 + sw],
                        start=(i == 0), stop=(i == NPR - 1))
                copy_out(Mt[mo][:, s0:s0 + sw], mps[:], mo * 2 + si)

        # ---- y = x @ M  (bf16 lhsT and rhs, fp32 psum) ----
        for m in range(NM):
            yt = work.tile([P, C], F32, tag="y", bufs=2, name=f"y{b}_{m}")
            for si, (s0, sw) in enumerate(SEGS):
                yps = ps.tile([P, sw], F32, tag="ps", padded_shape=[P, 512],
                              name=f"yps{b}_{m}_{si}", space="PSUM")
                for k in range(KC):
                    nc.tensor.matmul(
                        yps[:], xT[k][:, m * P:(m + 1) * P],
                        Mt[k][:, s0:s0 + sw],
                        start=(k == 0), stop=(k == KC - 1))
                copy_out(yt[:, s0:s0 + sw], yps[:], m * 2 + si)
                # per-segment DMA out: shorter tail after the last copy
                nc.sync.dma_start(y_d[b, m * P:(m + 1) * P, s0:s0 + sw],
                                  yt[:, s0:s0 + sw])


_BUILD_CACHE = {}


def build_program(mm_dt_name=MM_DT_NAME):
    if mm_dt_name in _BUILD_CACHE:
        return _BUILD_CACHE[mm_dt_name]
    mm_dt = F32R if mm_dt_name == "f32r" else F32
    nc = bacc.Bacc("TRN2", target_bir_lowering=False, debug=False,
                   num_devices=NCORES)
    x_d = nc.dram_tensor("x", [BS, N, C], F32, kind="ExternalInput").ap()
    wqkv_d = nc.dram_tensor("w_qkv", [C, 3 * C], F32, kind="ExternalInput").ap()
    wo_d = nc.dram_tensor("w_out", [C, C], F32, kind="ExternalInput").ap()
    y_d = nc.dram_tensor("y", [BS, N, C], F32, kind="ExternalOutput").ap()
    with tile.TileContext(nc) as tc:
        with ExitStack() as ctx:
            _emit(ctx, tc, mm_dt, x_d, wqkv_d, wo_d, y_d)
    nc.compile()
    _BUILD_CACHE[mm_dt_name] = nc
    return nc


def make_in_maps(x, w_qkv, w_out):
    x = np.ascontiguousarray(np.asarray(x, dtype=np.float32))
    w_qkv = np.ascontiguousarray(np.asarray(w_qkv, dtype=np.float32))
    w_out = np.ascontiguousarray(np.asarray(w_out, dtype=np.float32))
    return [
        {"x": x[i * BS:(i + 1) * BS], "w_qkv": w_qkv, "w_out": w_out}
        for i in range(NCORES)
    ]


def kernel(x, w_qkv, b_qkv=None, w_out=None, b_out=None, **_unused):
    nc = build_program()
    in_maps = make_in_maps(x, w_qkv, w_out)
    res = bass_utils.run_bass_kernel_spmd(nc, in_maps,
                                          core_ids=list(range(NCORES)))
    y = np.concatenate([res.results[i]["y"] for i in range(NCORES)], axis=0)
    return np.asarray(y, dtype=np.float32)
